# revision 3
# baseline (speedup 1.0000x reference)
"""Causal self-attention Trainium2 Bass kernel.

Problem: B=4, T=2048, D=1024, H=16, head_dim=64.
Sharding: 8 cores = (batch b in 0..3) x (head-group g in 0..1, 8 heads each).
Each core computes a partial projection output for its batch over its 512
model dims; the host sums the two partials per batch.

This environment charges a large fixed cost per *instruction* (matmul ~75us,
+~40us when the stationary operand changes, vector/act ~50us, DMA ~65us),
nearly independent of operand size, and dispatches all engines serially.
v3 therefore minimizes instruction count and stationary reloads:
  - qkv q/k loops reordered k-outer so each weight tile is loaded once and
    reused across 4 moving pieces (LDW 256 -> 64); evacuations merged to
    [128, t] copies (16 -> 8).
  - proj loops reordered m-outer within each query tile (LDW 128 -> 64);
    output staged in SBUF and written with ONE DMA (8 -> 1).
  - biases dropped from the device kernel (spec'd zero; host falls back to
    numpy if ever nonzero).
  - y^T overwrites the q^T storage in-place (q is dead once its head's
    scores are done), freeing SBUF for the single-DMA output staging.
  - esb pool bufs=8: longer slot-reuse distance elides ~54 WAR semaphore
    instructions (each costs dispatch time under serial issue).
"""

import numpy as np

import concourse.bacc as bacc
import concourse.bass as bass
import concourse.mybir as mybir
import concourse.tile as tile
from concourse.bass_utils import run_bass_kernel_spmd

F32 = mybir.dt.float32
F32R = mybir.dt.float32r
AF = mybir.ActivationFunctionType

B, T, D, H = 4, 2048, 1024, 16
HD = 64              # head dim
HPC = 8              # heads per core
DC = HPC * HD        # 512 model dims per core
SCALE = 1.0 / np.sqrt(HD)

_NC_CACHE = {}


def build_nc(t=T, reps=1):
    """Build the single-core SPMD program. t = sequence length (small sims)."""
    nt = t // 128          # 128-row tiles over time
    nq = t // 512          # 512-col chunks over time
    KC = D // 128          # 8 contraction chunks for qkv
    MQK = DC // 128        # 4 feature tiles for each of q,k

    nc = bacc.Bacc("TRN2", target_bir_lowering=False, debug=False)

    xT_d = nc.dram_tensor("xT", [D, t], F32R, kind="ExternalInput")
    wq_d = nc.dram_tensor("wq", [D, DC], F32R, kind="ExternalInput")
    wk_d = nc.dram_tensor("wk", [D, DC], F32R, kind="ExternalInput")
    wv_d = nc.dram_tensor("wv", [D, DC], F32R, kind="ExternalInput")
    wp_d = nc.dram_tensor("wp", [DC, D], F32R, kind="ExternalInput")
    out_d = nc.dram_tensor("out", [t, D], F32, kind="ExternalOutput")

    with tile.TileContext(nc) as tc:
      for _rep in range(reps):
        with tc.tile_pool(name="persist", bufs=1) as persist:

            # causal triangle mask, replicated 4x: trirep[p, j, u] = 1 iff
            # u >= p.  trirep[:, 0, :] masks a single diagonal 128-block;
            # trirep[:, 0:L, :] masks L stacked lanes in one op.
            trirep = persist.tile([128, 4, 128], F32)
            nc.gpsimd.memset(trirep[:], 1.0)
            nc.gpsimd.affine_select(
                out=trirep[:], in_=trirep[:],
                compare_op=mybir.AluOpType.is_ge, fill=0.0,
                base=0, pattern=[[0, 4], [1, 128]], channel_multiplier=-1)

            with tc.tile_pool(name="qkpool", bufs=1) as qkpool:
                # resident qk^T: [:, m, :] = q^T feats tile m, [:, 4+m, :] = k^T
                # (q^T of tile f is overwritten by y^T of heads 2f/2f+1 in B)
                qkTb = qkpool.tile([128, 2 * MQK, t], F32R)

                with tc.tile_pool(name="vpool", bufs=1) as vpool:
                    # v' mega-tile: [128, nt, 8*65]; col h*65+64 holds ones
                    vpm = vpool.tile([128, nt, HPC * (HD + 1)], F32R)
                    ones_f = persist.tile([128, nt * HPC], F32)
                    nc.gpsimd.memset(ones_f[:], 1.0)
                    nc.vector.tensor_copy(
                        vpm.rearrange("p t (h e) -> p (t h) e",
                                      e=HD + 1)[:, :, HD:HD + 1],
                        ones_f[:].unsqueeze(2))

                    # ---------------- Phase A: qkv ----------------
                    with tc.tile_pool(name="phA_sb", bufs=1) as pa, \
                         tc.tile_pool(name="phA_w", bufs=1) as pw, \
                         tc.tile_pool(name="phA_ps", bufs=1, space="PSUM") as pps:

                        # x^T resident: one tile [128, KC, t], single DMA
                        xTb = pa.tile([128, KC, t], F32R)
                        nc.sync.dma_start(
                            xTb[:], xT_d.rearrange("(k p) t -> p k t", p=128))

                        # q^T / k^T: k-outer, stationary reused over n pieces
                        for sec, w_d in enumerate((wq_d, wk_d)):
                            ws = pw.tile([128, KC, DC], F32R, name=f"ws{sec}",
                                         tag="wsec")
                            nc.sync.dma_start(
                                ws[:], w_d.rearrange("(k p) c -> p k c", p=128))
                            for mg in range(MQK // 2):
                                psm = [pps.tile([128, t], F32, name=f"qkps{m2}",
                                                tag=f"psqk{m2}")
                                       for m2 in range(2)]
                                for k in range(KC):
                                    for m2 in range(2):
                                        m = mg * 2 + m2
                                        for n in range(nq):
                                            nc.tensor.matmul(
                                                psm[m2][:, n * 512:(n + 1) * 512],
                                                ws[:, k, m * 128:(m + 1) * 128],
                                                xTb[:, k, n * 512:(n + 1) * 512],
                                                start=(k == 0),
                                                stop=(k == KC - 1))
                                for m2 in range(2):
                                    nc.vector.tensor_copy(
                                        qkTb[:, sec * MQK + mg * 2 + m2, :],
                                        psm[m2][:])

                        # v natural -> strided copy into v' tiles
                        wvs = pw.tile([128, KC, DC], F32R, name="wvs",
                                      tag="wsec")
                        nc.sync.dma_start(
                            wvs[:], wv_d.rearrange("(k p) c -> p k c", p=128))
                        for tt in range(nt):
                            ps = pps.tile([128, DC], F32, name="vps",
                                          tag="psqk0")
                            for k in range(KC):
                                nc.tensor.matmul(
                                    ps[:],
                                    xTb[:, k, tt * 128:(tt + 1) * 128],
                                    wvs[:, k, :],
                                    start=(k == 0), stop=(k == KC - 1))
                            nc.vector.tensor_copy(
                                vpm[:, tt].rearrange("p (h e) -> p h e",
                                                     e=HD + 1)[:, :, 0:HD],
                                ps.rearrange("p (h e) -> p h e", e=HD))

                    # ---------------- Phase B: attention ----------------
                    # kc chunks grouped per 512-diagonal band so several
                    # chunks share one scores-psum tile, one exp, one mask:
                    # lane width W = t - dn*512, L = lanes per 4-bank tile.
                    groups = []
                    for dn in range(nq):
                        W = t - dn * 512
                        L = min(4, max(1, 2048 // W))
                        band = list(range(4 * dn, 4 * dn + 4))
                        for i in range(0, 4, L):
                            groups.append((dn, W, band[i:i + L]))

                    with tc.tile_pool(name="esb", bufs=8) as pesb, \
                         tc.tile_pool(name="norm", bufs=1) as pnorm, \
                         tc.tile_pool(name="gmask", bufs=1) as pgm, \
                         tc.tile_pool(name="sc_ps", bufs=1, space="PSUM") as pscps, \
                         tc.tile_pool(name="y_ps", bufs=1, space="PSUM") as pyps:

                        # per-group full-width masks (1 iff query >= key):
                        # gm[p, j, u] = 1 iff (w_start+u) >= (dlo_j + p),
                        # dlo_j = 128*lanes[j] = w_start + off0 + 128*j
                        gmasks = {}
                        for gi, (dn, W, lanes) in enumerate(groups):
                            if len(lanes) == 1:
                                continue
                            L = len(lanes)
                            off0 = 128 * lanes[0] - dn * 512
                            gm = pgm.tile([128, L, W], F32, name=f"gm{gi}",
                                          tag=f"gm{gi}")
                            nc.gpsimd.memset(
                                gm.rearrange("p l w -> p (l w)"), 1.0)
                            nc.gpsimd.affine_select(
                                out=gm[:], in_=gm[:],
                                compare_op=mybir.AluOpType.is_ge, fill=0.0,
                                base=-off0, pattern=[[-128, L], [1, W]],
                                channel_multiplier=-1)
                            gmasks[gi] = gm

                        for f in range(MQK):
                            for hh in range(2):
                                h = 2 * f + hh
                                qh = qkTb[:, f][hh * HD:(hh + 1) * HD, :]
                                kh = qkTb[:, MQK + f][hh * HD:(hh + 1) * HD, :]
                                y_acc = pyps.tile([HD + 1, t], F32,
                                                  name=f"yacc{h}", tag="yacc")
                                for gi, (dn, W, lanes) in enumerate(groups):
                                    L = len(lanes)
                                    ws_ = dn * 512    # group query-window start
                                    if L == 1:
                                        kc = lanes[0]
                                        dlo = 128 * kc
                                        sp = pscps.tile([128, t], F32,
                                                        name="scps", tag="scps")
                                        for n in range(dn, nq):
                                            w0 = dlo if n == dn else n * 512
                                            nc.tensor.matmul(
                                                sp[:, w0:(n + 1) * 512],
                                                kh[:, kc * 128:(kc + 1) * 128],
                                                qh[:, w0:(n + 1) * 512],
                                                start=True, stop=True)
                                        esb = pesb.tile([128, t], F32R,
                                                        name="esb", tag="esb")
                                        nc.scalar.activation(
                                            esb[:, dlo:], sp[:, dlo:],
                                            AF.Exp, scale=float(SCALE))
                                        nc.vector.tensor_mul(
                                            esb[:, dlo:dlo + 128],
                                            esb[:, dlo:dlo + 128],
                                            trirep[:, 0, :])
                                        for n in range(dn, nq):
                                            w0 = dlo if n == dn else n * 512
                                            nc.tensor.matmul(
                                                y_acc[:, w0:(n + 1) * 512],
                                                vpm[:, kc,
                                                    h * (HD + 1):(h + 1) * (HD + 1)],
                                                esb[:, w0:(n + 1) * 512],
                                                start=(kc == 0),
                                                stop=(kc == 4 * n + 3))
                                    else:
                                        sp = pscps.tile([128, L, W], F32,
                                                        name="scps", tag="scps")
                                        for j, kc in enumerate(lanes):
                                            for n in range(dn, nq):
                                                w0 = ws_ if n == dn else n * 512
                                                nc.tensor.matmul(
                                                    sp[:, j,
                                                       w0 - ws_:(n + 1) * 512 - ws_],
                                                    kh[:, kc * 128:(kc + 1) * 128],
                                                    qh[:, w0:(n + 1) * 512],
                                                    start=True, stop=True)
                                        esb = pesb.tile([128, L * W], F32R,
                                                        name="esb", tag="esb")
                                        nc.scalar.activation(
                                            esb[:, 0:L * W],
                                            sp.rearrange("p l w -> p (l w)"),
                                            AF.Exp, scale=float(SCALE))
                                        nc.vector.tensor_mul(
                                            esb[:, 0:L * W],
                                            esb[:, 0:L * W],
                                            gmasks[gi].rearrange(
                                                "p l w -> p (l w)"))
                                        for j, kc in enumerate(lanes):
                                            dlo = 128 * kc
                                            for n in range(dn, nq):
                                                w0 = dlo if n == dn else n * 512
                                                nc.tensor.matmul(
                                                    y_acc[:, w0:(n + 1) * 512],
                                                    vpm[:, kc,
                                                        h * (HD + 1):(h + 1) * (HD + 1)],
                                                    esb[:, j * W + w0 - ws_:
                                                         j * W + (n + 1) * 512 - ws_],
                                                    start=(kc == 0),
                                                    stop=(kc == 4 * n + 3))
                                # normalize: y = y_raw / denom.  hh=0 lands in
                                # a temp (its q rows are still live for hh=1);
                                # hh=1 overwrites its own dead q rows directly.
                                rec = pnorm.tile([1, t], F32, name="rec",
                                                 tag="rec")
                                nc.vector.reciprocal(rec[:],
                                                     y_acc[HD:HD + 1, :])
                                rb = pnorm.tile([HD, t], F32, name="rb",
                                                tag="rb")
                                nc.gpsimd.partition_broadcast(rb[:], rec[:])
                                nc.vector.tensor_mul(
                                    qkTb[:, f][hh * HD:(hh + 1) * HD, :],
                                    y_acc[0:HD, :], rb[:])

            # ---------------- Phase C: projection ----------------
            # yT[f] now lives in qkTb[:, f]
                with tc.tile_pool(name="phC_sb", bufs=1) as pc, \
                     tc.tile_pool(name="phC_ps", bufs=2, space="PSUM") as pcps:
                    wpb = pc.tile([128, MQK, D], F32R)
                    nc.sync.dma_start(
                        wpb[:], wp_d.rearrange("(m p) o -> p m o", p=128))
                    stage = pc.tile([128, nt, D], F32)
                    for qt in range(nt):
                        ps = pcps.tile([128, D], F32, name="prps", tag="prps")
                        for m in range(MQK):
                            for oc in range(D // 512):
                                nc.tensor.matmul(
                                    ps[:, oc * 512:(oc + 1) * 512],
                                    qkTb[:, m][:, qt * 128:(qt + 1) * 128],
                                    wpb[:, m, oc * 512:(oc + 1) * 512],
                                    start=(m == 0), stop=(m == MQK - 1))
                        nc.vector.tensor_copy(stage[:, qt, :], ps[:])
                    nc.sync.dma_start(
                        out_d.rearrange("(a p) o -> p a o", p=128),
                        stage[:])

    nc.finalize()
    return nc


def make_in_maps(x, w_attn, b_attn, w_proj, b_proj):
    x = np.ascontiguousarray(np.asarray(x, dtype=np.float32))
    w_attn = np.asarray(w_attn, dtype=np.float32)
    w_proj = np.asarray(w_proj, dtype=np.float32)
    xTs = [np.ascontiguousarray(x[b].T) for b in range(B)]
    in_maps = []
    for c in range(8):
        b, g = c // 2, c % 2
        sl = slice(DC * g, DC * (g + 1))
        in_maps.append({
            "xT": xTs[b],
            "wq": np.ascontiguousarray(w_attn[:, 0 * D:][:, sl]),
            "wk": np.ascontiguousarray(w_attn[:, 1 * D:][:, sl]),
            "wv": np.ascontiguousarray(w_attn[:, 2 * D:][:, sl]),
            "wp": np.ascontiguousarray(w_proj[sl, :]),
        })
    return in_maps


def _numpy_fallback(x, w_attn, b_attn, w_proj, b_proj):
    """Reference path for nonzero biases (never taken for the spec'd inputs)."""
    x = np.asarray(x, dtype=np.float32)
    qkv = x @ w_attn + b_attn
    q, k, v = np.split(qkv, 3, axis=-1)
    Bv, Tv, Dv = x.shape
    q = q.reshape(Bv, Tv, H, HD).transpose(0, 2, 1, 3)
    k = k.reshape(Bv, Tv, H, HD).transpose(0, 2, 1, 3)
    v = v.reshape(Bv, Tv, H, HD).transpose(0, 2, 1, 3)
    s = np.einsum("bhqd,bhkd->bhqk", q, k) / np.sqrt(HD)
    mask = np.tril(np.ones((Tv, Tv), dtype=bool))
    s = np.where(mask, s, -np.inf)
    s -= s.max(axis=-1, keepdims=True)
    p = np.exp(s)
    p /= p.sum(axis=-1, keepdims=True)
    y = np.einsum("bhqk,bhkd->bhqd", p, v)
    y = y.transpose(0, 2, 1, 3).reshape(Bv, Tv, Dv)
    return (y @ w_proj + b_proj).astype(np.float32)


def kernel(x, w_attn, b_attn, w_proj, b_proj, _trace=False, _trace_kwargs=None):
    b_attn = np.asarray(b_attn, dtype=np.float32)
    b_proj = np.asarray(b_proj, dtype=np.float32)
    if np.any(b_attn) or np.any(b_proj):
        return _numpy_fallback(x, w_attn, b_attn, w_proj, b_proj)
    if "nc" not in _NC_CACHE:
        _NC_CACHE["nc"] = build_nc()
    nc = _NC_CACHE["nc"]
    in_maps = make_in_maps(x, w_attn, b_attn, w_proj, b_proj)
    kw = {}
    if _trace:
        kw["trace"] = True
        if _trace_kwargs:
            kw.update(_trace_kwargs)
    res = run_bass_kernel_spmd(nc, in_maps, core_ids=list(range(8)), **kw)
    outs = [res.results[c]["out"] for c in range(8)]
    out = np.empty((B, T, D), dtype=np.float32)
    for b in range(B):
        np.add(outs[2 * b], outs[2 * b + 1], out=out[b])
    kernel._last_results = res
    return out


if __name__ == "__main__":
    nc = build_nc()
    print("built ok")


# revision 4
# speedup vs baseline: 1.5728x; 1.5728x over previous
"""Causal self-attention Trainium2 Bass kernel.

Problem: B=4, T=2048, D=1024, H=16, head_dim=64.
Sharding: 8 cores = (batch b in 0..3) x (head-group g in 0..1, 8 heads each).
Each core computes a partial projection output for its batch over its 512
model dims; the host sums the two partials per batch.

This environment charges a large fixed cost per *instruction* (matmul ~75us,
+~40us when the stationary operand changes, vector/act ~50us, DMA ~65us),
nearly independent of operand size, and dispatches all engines serially.
v3 therefore minimizes instruction count and stationary reloads:
  - qkv q/k loops reordered k-outer so each weight tile is loaded once and
    reused across 4 moving pieces (LDW 256 -> 64); evacuations merged to
    [128, t] copies (16 -> 8).
  - proj loops reordered m-outer within each query tile (LDW 128 -> 64);
    output staged in SBUF and written with ONE DMA (8 -> 1).
  - biases dropped from the device kernel (spec'd zero; host falls back to
    numpy if ever nonzero).
  - y^T overwrites the q^T storage in-place (q is dead once its head's
    scores are done), freeing SBUF for the single-DMA output staging.
  - esb pool bufs=8: longer slot-reuse distance elides ~54 WAR semaphore
    instructions (each costs dispatch time under serial issue).
"""

import numpy as np

import concourse.bacc as bacc
import concourse.bass as bass
import concourse.mybir as mybir
import concourse.tile as tile
from concourse.bass_utils import run_bass_kernel_spmd

F32 = mybir.dt.float32
F32R = mybir.dt.float32r
AF = mybir.ActivationFunctionType

B, T, D, H = 4, 2048, 1024, 16
HD = 64              # head dim
HPC = 8              # heads per core
DC = HPC * HD        # 512 model dims per core
SCALE = 1.0 / np.sqrt(HD)

_NC_CACHE = {}


def build_nc(t=T, reps=1):
    """Build the single-core SPMD program. t = sequence length (small sims)."""
    nt = t // 128          # 128-row tiles over time
    nq = t // 512          # 512-col chunks over time
    KC = D // 128          # 8 contraction chunks for qkv
    MQK = DC // 128        # 4 feature tiles for each of q,k

    nc = bacc.Bacc("TRN2", target_bir_lowering=False, debug=False)

    xT_d = nc.dram_tensor("xT", [D, t], F32R, kind="ExternalInput")
    wq_d = nc.dram_tensor("wq", [D, DC], F32R, kind="ExternalInput")
    wk_d = nc.dram_tensor("wk", [D, DC], F32R, kind="ExternalInput")
    wv_d = nc.dram_tensor("wv", [D, DC], F32R, kind="ExternalInput")
    wp_d = nc.dram_tensor("wp", [DC, D], F32R, kind="ExternalInput")
    out_d = nc.dram_tensor("out", [t, D], F32, kind="ExternalOutput")

    with tile.TileContext(nc) as tc:
      for _rep in range(reps):
        with tc.tile_pool(name="persist", bufs=1) as persist:

            # causal triangle mask, replicated 4x: trirep[p, j, u] = 1 iff
            # u >= p.  trirep[:, 0, :] masks a single diagonal 128-block;
            # trirep[:, 0:L, :] masks L stacked lanes in one op.
            trirep = persist.tile([128, 4, 128], F32)
            nc.gpsimd.memset(trirep[:], 1.0)
            nc.gpsimd.affine_select(
                out=trirep[:], in_=trirep[:],
                compare_op=mybir.AluOpType.is_ge, fill=0.0,
                base=0, pattern=[[0, 4], [1, 128]], channel_multiplier=-1)

            with tc.tile_pool(name="qkpool", bufs=1) as qkpool:
                # resident qk^T: [:, m, :] = q^T feats tile m, [:, 4+m, :] = k^T
                # (q^T of tile f is overwritten by y^T of heads 2f/2f+1 in B)
                qkTb = qkpool.tile([128, 2 * MQK, t], F32R)

                with tc.tile_pool(name="vpool", bufs=1) as vpool:
                    # v' mega-tile: [128, nt, 8*65]; col h*65+64 holds ones
                    vpm = vpool.tile([128, nt, HPC * (HD + 1)], F32R)
                    ones_f = persist.tile([128, nt * HPC], F32)
                    nc.gpsimd.memset(ones_f[:], 1.0)
                    nc.vector.tensor_copy(
                        vpm.rearrange("p t (h e) -> p (t h) e",
                                      e=HD + 1)[:, :, HD:HD + 1],
                        ones_f[:].unsqueeze(2))

                    # ---------------- Phase A: qkv ----------------
                    with tc.tile_pool(name="phA_sb", bufs=1) as pa, \
                         tc.tile_pool(name="phA_w", bufs=1) as pw, \
                         tc.tile_pool(name="phA_ps", bufs=1, space="PSUM") as pps:

                        # x^T resident: one tile [128, KC, t], single DMA
                        xTb = pa.tile([128, KC, t], F32R)
                        nc.sync.dma_start(
                            xTb[:], xT_d.rearrange("(k p) t -> p k t", p=128))

                        # q^T / k^T: k-outer, stationary reused over n pieces
                        for sec, w_d in enumerate((wq_d, wk_d)):
                            ws = pw.tile([128, KC, DC], F32R, name=f"ws{sec}",
                                         tag="wsec")
                            nc.sync.dma_start(
                                ws[:], w_d.rearrange("(k p) c -> p k c", p=128))
                            for mg in range(MQK // 2):
                                psm = [pps.tile([128, t], F32, name=f"qkps{m2}",
                                                tag=f"psqk{m2}")
                                       for m2 in range(2)]
                                for k in range(KC):
                                    for m2 in range(2):
                                        m = mg * 2 + m2
                                        for n in range(nq):
                                            nc.tensor.matmul(
                                                psm[m2][:, n * 512:(n + 1) * 512],
                                                ws[:, k, m * 128:(m + 1) * 128],
                                                xTb[:, k, n * 512:(n + 1) * 512],
                                                start=(k == 0),
                                                stop=(k == KC - 1))
                                for m2 in range(2):
                                    nc.vector.tensor_copy(
                                        qkTb[:, sec * MQK + mg * 2 + m2, :],
                                        psm[m2][:])

                        # v natural -> strided copy into v' tiles
                        wvs = pw.tile([128, KC, DC], F32R, name="wvs",
                                      tag="wsec")
                        nc.sync.dma_start(
                            wvs[:], wv_d.rearrange("(k p) c -> p k c", p=128))
                        for tt in range(nt):
                            ps = pps.tile([128, DC], F32, name="vps",
                                          tag="psqk0")
                            for k in range(KC):
                                nc.tensor.matmul(
                                    ps[:],
                                    xTb[:, k, tt * 128:(tt + 1) * 128],
                                    wvs[:, k, :],
                                    start=(k == 0), stop=(k == KC - 1))
                            nc.vector.tensor_copy(
                                vpm[:, tt].rearrange("p (h e) -> p h e",
                                                     e=HD + 1)[:, :, 0:HD],
                                ps.rearrange("p (h e) -> p h e", e=HD))

                    # ---------------- Phase B: attention ----------------
                    # kc chunks grouped per 512-diagonal band so several
                    # chunks share one scores-psum tile, one exp, one mask:
                    # lane width W = t - dn*512, L = lanes per 4-bank tile.
                    groups = []
                    for dn in range(nq):
                        W = t - dn * 512
                        L = min(4, max(1, 2048 // W))
                        band = list(range(4 * dn, 4 * dn + 4))
                        for i in range(0, 4, L):
                            groups.append((dn, W, band[i:i + L]))

                    with tc.tile_pool(name="esb", bufs=8) as pesb, \
                         tc.tile_pool(name="norm", bufs=1) as pnorm, \
                         tc.tile_pool(name="gmask", bufs=1) as pgm, \
                         tc.tile_pool(name="sc_ps", bufs=1, space="PSUM") as pscps, \
                         tc.tile_pool(name="y_ps", bufs=1, space="PSUM") as pyps:

                        # per-group full-width masks (1 iff query >= key):
                        # gm[p, j, u] = 1 iff (w_start+u) >= (dlo_j + p),
                        # dlo_j = 128*lanes[j] = w_start + off0 + 128*j
                        gmasks = {}
                        for gi, (dn, W, lanes) in enumerate(groups):
                            if len(lanes) == 1:
                                continue
                            L = len(lanes)
                            off0 = 128 * lanes[0] - dn * 512
                            gm = pgm.tile([128, L, W], F32, name=f"gm{gi}",
                                          tag=f"gm{gi}")
                            nc.gpsimd.memset(
                                gm.rearrange("p l w -> p (l w)"), 1.0)
                            nc.gpsimd.affine_select(
                                out=gm[:], in_=gm[:],
                                compare_op=mybir.AluOpType.is_ge, fill=0.0,
                                base=-off0, pattern=[[-128, L], [1, W]],
                                channel_multiplier=-1)
                            gmasks[gi] = gm

                        for f in range(MQK):
                            for hh in range(2):
                                h = 2 * f + hh
                                qh = qkTb[:, f][hh * HD:(hh + 1) * HD, :]
                                kh = qkTb[:, MQK + f][hh * HD:(hh + 1) * HD, :]
                                y_acc = pyps.tile([HD + 1, t], F32,
                                                  name=f"yacc{h}", tag="yacc")
                                for gi, (dn, W, lanes) in enumerate(groups):
                                    L = len(lanes)
                                    ws_ = dn * 512    # group query-window start
                                    if L == 1:
                                        kc = lanes[0]
                                        dlo = 128 * kc
                                        sp = pscps.tile([128, t], F32,
                                                        name="scps", tag="scps")
                                        for n in range(dn, nq):
                                            w0 = dlo if n == dn else n * 512
                                            nc.tensor.matmul(
                                                sp[:, w0:(n + 1) * 512],
                                                kh[:, kc * 128:(kc + 1) * 128],
                                                qh[:, w0:(n + 1) * 512],
                                                start=True, stop=True)
                                        esb = pesb.tile([128, t], F32R,
                                                        name="esb", tag="esb")
                                        nc.scalar.activation(
                                            esb[:, dlo:], sp[:, dlo:],
                                            AF.Exp, scale=float(SCALE))
                                        nc.vector.tensor_mul(
                                            esb[:, dlo:dlo + 128],
                                            esb[:, dlo:dlo + 128],
                                            trirep[:, 0, :])
                                        for n in range(dn, nq):
                                            w0 = dlo if n == dn else n * 512
                                            nc.tensor.matmul(
                                                y_acc[:, w0:(n + 1) * 512],
                                                vpm[:, kc,
                                                    h * (HD + 1):(h + 1) * (HD + 1)],
                                                esb[:, w0:(n + 1) * 512],
                                                start=(kc == 0),
                                                stop=(kc == 4 * n + 3))
                                    else:
                                        sp = pscps.tile([128, L, W], F32,
                                                        name="scps", tag="scps")
                                        for j, kc in enumerate(lanes):
                                            for n in range(dn, nq):
                                                w0 = ws_ if n == dn else n * 512
                                                nc.tensor.matmul(
                                                    sp[:, j,
                                                       w0 - ws_:(n + 1) * 512 - ws_],
                                                    kh[:, kc * 128:(kc + 1) * 128],
                                                    qh[:, w0:(n + 1) * 512],
                                                    start=True, stop=True)
                                        esb = pesb.tile([128, L * W], F32R,
                                                        name="esb", tag="esb")
                                        nc.scalar.activation(
                                            esb[:, 0:L * W],
                                            sp.rearrange("p l w -> p (l w)"),
                                            AF.Exp, scale=float(SCALE))
                                        nc.vector.tensor_mul(
                                            esb[:, 0:L * W],
                                            esb[:, 0:L * W],
                                            gmasks[gi].rearrange(
                                                "p l w -> p (l w)"))
                                        for j, kc in enumerate(lanes):
                                            dlo = 128 * kc
                                            for n in range(dn, nq):
                                                w0 = dlo if n == dn else n * 512
                                                nc.tensor.matmul(
                                                    y_acc[:, w0:(n + 1) * 512],
                                                    vpm[:, kc,
                                                        h * (HD + 1):(h + 1) * (HD + 1)],
                                                    esb[:, j * W + w0 - ws_:
                                                         j * W + (n + 1) * 512 - ws_],
                                                    start=(kc == 0),
                                                    stop=(kc == 4 * n + 3))
                                # normalize: y = y_raw / denom.  hh=0 lands in
                                # a temp (its q rows are still live for hh=1);
                                # hh=1 overwrites its own dead q rows directly.
                                rec = pnorm.tile([1, t], F32, name="rec",
                                                 tag="rec")
                                nc.vector.reciprocal(rec[:],
                                                     y_acc[HD:HD + 1, :])
                                rb = pnorm.tile([HD, t], F32, name="rb",
                                                tag="rb")
                                nc.gpsimd.partition_broadcast(rb[:], rec[:])
                                nc.vector.tensor_mul(
                                    qkTb[:, f][hh * HD:(hh + 1) * HD, :],
                                    y_acc[0:HD, :], rb[:])

            # ---------------- Phase C: projection ----------------
            # yT[f] now lives in qkTb[:, f]
                with tc.tile_pool(name="phC_sb", bufs=1) as pc, \
                     tc.tile_pool(name="phC_ps", bufs=2, space="PSUM") as pcps:
                    wpb = pc.tile([128, MQK, D], F32R)
                    nc.sync.dma_start(
                        wpb[:], wp_d.rearrange("(m p) o -> p m o", p=128))
                    stage = pc.tile([128, nt, D], F32)
                    for qtp in range(nt // 2):
                        ps = pcps.tile([128, 2, D], F32, name="prps",
                                       tag="prps")
                        for half in range(2):
                            qt = 2 * qtp + half
                            for m in range(MQK):
                                for oc in range(D // 512):
                                    nc.tensor.matmul(
                                        ps[:, half, oc * 512:(oc + 1) * 512],
                                        qkTb[:, m][:, qt * 128:(qt + 1) * 128],
                                        wpb[:, m, oc * 512:(oc + 1) * 512],
                                        start=(m == 0), stop=(m == MQK - 1))
                        nc.vector.tensor_copy(
                            stage[:, 2 * qtp:2 * qtp + 2, :]
                            .rearrange("p a o -> p (a o)"),
                            ps.rearrange("p a o -> p (a o)"))
                    nc.sync.dma_start(
                        out_d.rearrange("(a p) o -> p a o", p=128),
                        stage[:])

    nc.finalize()
    return nc


def make_in_maps(x, w_attn, b_attn, w_proj, b_proj):
    x = np.ascontiguousarray(np.asarray(x, dtype=np.float32))
    w_attn = np.asarray(w_attn, dtype=np.float32)
    w_proj = np.asarray(w_proj, dtype=np.float32)
    xTs = [np.ascontiguousarray(x[b].T) for b in range(B)]
    in_maps = []
    for c in range(8):
        b, g = c // 2, c % 2
        sl = slice(DC * g, DC * (g + 1))
        in_maps.append({
            "xT": xTs[b],
            "wq": np.ascontiguousarray(w_attn[:, 0 * D:][:, sl]),
            "wk": np.ascontiguousarray(w_attn[:, 1 * D:][:, sl]),
            "wv": np.ascontiguousarray(w_attn[:, 2 * D:][:, sl]),
            "wp": np.ascontiguousarray(w_proj[sl, :]),
        })
    return in_maps


def _numpy_fallback(x, w_attn, b_attn, w_proj, b_proj):
    """Reference path for nonzero biases (never taken for the spec'd inputs)."""
    x = np.asarray(x, dtype=np.float32)
    qkv = x @ w_attn + b_attn
    q, k, v = np.split(qkv, 3, axis=-1)
    Bv, Tv, Dv = x.shape
    q = q.reshape(Bv, Tv, H, HD).transpose(0, 2, 1, 3)
    k = k.reshape(Bv, Tv, H, HD).transpose(0, 2, 1, 3)
    v = v.reshape(Bv, Tv, H, HD).transpose(0, 2, 1, 3)
    s = np.einsum("bhqd,bhkd->bhqk", q, k) / np.sqrt(HD)
    mask = np.tril(np.ones((Tv, Tv), dtype=bool))
    s = np.where(mask, s, -np.inf)
    s -= s.max(axis=-1, keepdims=True)
    p = np.exp(s)
    p /= p.sum(axis=-1, keepdims=True)
    y = np.einsum("bhqk,bhkd->bhqd", p, v)
    y = y.transpose(0, 2, 1, 3).reshape(Bv, Tv, Dv)
    return (y @ w_proj + b_proj).astype(np.float32)


def kernel(x, w_attn, b_attn, w_proj, b_proj, _trace=False, _trace_kwargs=None):
    b_attn = np.asarray(b_attn, dtype=np.float32)
    b_proj = np.asarray(b_proj, dtype=np.float32)
    if np.any(b_attn) or np.any(b_proj):
        return _numpy_fallback(x, w_attn, b_attn, w_proj, b_proj)
    if "nc" not in _NC_CACHE:
        _NC_CACHE["nc"] = build_nc()
    nc = _NC_CACHE["nc"]
    in_maps = make_in_maps(x, w_attn, b_attn, w_proj, b_proj)
    kw = {}
    if _trace:
        kw["trace"] = True
        if _trace_kwargs:
            kw.update(_trace_kwargs)
    res = run_bass_kernel_spmd(nc, in_maps, core_ids=list(range(8)), **kw)
    outs = [res.results[c]["out"] for c in range(8)]
    out = np.empty((B, T, D), dtype=np.float32)
    for b in range(B):
        np.add(outs[2 * b], outs[2 * b + 1], out=out[b])
    kernel._last_results = res
    return out


if __name__ == "__main__":
    nc = build_nc()
    print("built ok")
